# revision 12
# baseline (speedup 1.0000x reference)
"""Channel-attention Trainium2 kernel (Bass/Tile, 8 NeuronCores).

The reference computes, after un-permuting the V path:

    out[b,c,t,f] = sum_k w[b, f//64, c, k] * x[b,k,t,f]
    w[b,h]       = softmax_k( (q_h q-rows) @ (k_h rows)^T / 8 )
    q            = mean_t(x[b]) @ Wq.T + bq,   k = mean_t(x[b]) @ Wk.T

i.e. a per-(batch, head) 128x128 channel-mixing matmul over the full
(T x 64) feature block, fed by a tiny pooled q/k path.

Sharding: 8 cores = (batch b in {0,1}) x (T-quarter q in {0..3}); each core
owns x[b, :, q*128:(q+1)*128, :] (32 MB, fully contiguous per-channel rows).

Phase 1 (device): per-core partial sum over its t-slice -> (128, 512).
Host: combine 4 partials per batch into xm[b] (tiny, 256 KB).
Phase 2 (device): per-core replicated pooled path (q/k projections, per-head
softmax, transpose) then the streaming per-head matmuls with contiguous
8KB-per-partition DMAs in both directions.
"""

import numpy as np

import concourse.bacc as bacc
import concourse.mybir as mybir
import concourse.tile as tile
from concourse.bass import ds, ts
from concourse.bass_utils import run_bass_kernel_spmd
from concourse.masks import make_identity

B, C, T, F = 2, 128, 512, 512
H = 8
D = F // H            # 64 features per head
NCORES = 8
TQ = T // 4           # 128 t's per core
FCHUNKS = F // 128    # 4
F32 = mybir.dt.float32

# test.py can flip this to get NTFF profiling / exec_time_ns
TRACE = False
LAST_PROFILE = {}

_CACHE = {}


def _make_nc():
    return bacc.Bacc(
        "TRN2", target_bir_lowering=False, debug=False, num_devices=NCORES
    )


def _build_phase1(repeat=1):
    """Partial sum over the core's 128 t's: ps[c, f] = sum_t xs[c, t, f].

    repeat>1 re-runs the streaming pass (same reads) for benchmarking only.
    """
    nc = _make_nc()
    xs = nc.dram_tensor("xs", [C, TQ, F], F32, kind="ExternalInput")
    ps = nc.dram_tensor("ps", [C, F], F32, kind="ExternalOutput")
    TT = 8
    with tile.TileContext(nc) as tc:
        with (
            tc.tile_pool(name="xin", bufs=3) as xpool,
            tc.tile_pool(name="tmp", bufs=2) as tpool,
            tc.tile_pool(name="accp", bufs=1) as apool,
        ):
            acc = apool.tile([C, F], F32, name="acc")
            for rep in range(repeat):
                for it in range(TQ // TT):
                    xt = xpool.tile([C, TT, F], F32, name="xt")
                    # alternate the two HWDGE rings (SP / ACT) for read BW
                    eng = nc.sync if it % 2 == 0 else nc.scalar
                    eng.dma_start(xt[:], xs[:, ts(it, TT), :])
                    view = xt[:].rearrange("c t f -> c f t")
                    if rep == 0 and it == 0:
                        nc.vector.reduce_sum(acc[:], view, axis=mybir.AxisListType.X)
                    else:
                        red = tpool.tile([C, F], F32, name="red")
                        nc.vector.reduce_sum(red[:], view, axis=mybir.AxisListType.X)
                        nc.vector.tensor_add(acc[:], acc[:], red[:])
            nc.sync.dma_start(ps[:], acc[:])
    nc.finalize()
    return nc


def _build_phase2(repeat=1):
    """Pooled q/k path (replicated per core) + streaming per-head matmuls.

    repeat>1 re-runs the streaming pass (same reads/writes), bench only.
    """
    nc = _make_nc()
    xs = nc.dram_tensor("xs", [C, TQ, F], F32, kind="ExternalInput")     # (k,t,f)
    xmt = nc.dram_tensor("xmt", [F, C], F32, kind="ExternalInput")       # xm[b].T
    wqt = nc.dram_tensor("wqt", [F, F], F32, kind="ExternalInput")       # Wq.T (f,j)
    wkt = nc.dram_tensor("wkt", [F, F], F32, kind="ExternalInput")       # Wk.T (f,j)
    bqr = nc.dram_tensor("bqr", [C, FCHUNKS], F32, kind="ExternalInput")  # bq (4,128).T
    out = nc.dram_tensor("out", [C, TQ, F], F32, kind="ExternalOutput")  # (c,t,f)
    TT = 8  # t's per DMA tile (2 MB transfers)
    with tile.TileContext(nc) as tc:
        with (
            tc.tile_pool(name="const", bufs=1) as const,
            tc.tile_pool(name="wts", bufs=1) as wts,
            tc.tile_pool(name="small", bufs=2) as small,
            tc.tile_pool(name="xin", bufs=3) as xpool,
            tc.tile_pool(name="oout", bufs=3) as opool,
            tc.tile_pool(name="pqk", bufs=2, space="PSUM") as pqk,
            tc.tile_pool(name="pbig", bufs=6, space="PSUM") as pbig,
        ):
            ident = const.tile([128, 128], F32, name="ident")
            make_identity(nc, ident)
            wqt_sb = const.tile([128, FCHUNKS, F], F32, name="wqt_sb")
            nc.sync.dma_start(wqt_sb[:], wqt.rearrange("(o p) j -> p o j", p=128))
            wkt_sb = const.tile([128, FCHUNKS, F], F32, name="wkt_sb")
            nc.sync.dma_start(wkt_sb[:], wkt.rearrange("(o p) j -> p o j", p=128))
            xmt_sb = const.tile([128, FCHUNKS, C], F32, name="xmt_sb")
            nc.sync.dma_start(xmt_sb[:], xmt.rearrange("(o p) c -> p o c", p=128))
            bq_sb = const.tile([C, FCHUNKS], F32, name="bq_sb")
            nc.sync.dma_start(bq_sb[:], bqr[:])

            # qT[j, c] = sum_f Wq[j, f] xm[c, f] + bq[j]; kT likewise (no bias).
            qt_sb = wts.tile([128, FCHUNKS, C], F32, name="qt_sb")
            kt_sb = wts.tile([128, FCHUNKS, C], F32, name="kt_sb")
            for jc in range(FCHUNKS):
                psq = pqk.tile([128, C], F32, name="psq", tag="smallps")
                for fc in range(FCHUNKS):
                    nc.tensor.matmul(
                        psq[:],
                        wqt_sb[:, fc, ts(jc, 128)],
                        xmt_sb[:, fc, :],
                        start=(fc == 0),
                        stop=(fc == FCHUNKS - 1),
                    )
                nc.scalar.activation(
                    qt_sb[:, jc, :],
                    psq[:],
                    mybir.ActivationFunctionType.Identity,
                    bias=bq_sb[:, jc : jc + 1],
                    scale=1.0,
                )
                psk = pqk.tile([128, C], F32, name="psk", tag="smallps")
                for fc in range(FCHUNKS):
                    nc.tensor.matmul(
                        psk[:],
                        wkt_sb[:, fc, ts(jc, 128)],
                        xmt_sb[:, fc, :],
                        start=(fc == 0),
                        stop=(fc == FCHUNKS - 1),
                    )
                nc.scalar.copy(kt_sb[:, jc, :], psk[:])

            # Per-head attention weights, stored transposed: wT[k2, h, c].
            wt_sb = wts.tile([128, H, C], F32, name="wt_sb")
            for h in range(H):
                jc, off = h // 2, D * (h % 2)
                psa = pqk.tile([C, C], F32, name="psa", tag="smallps")
                nc.tensor.matmul(
                    psa[:],
                    qt_sb[off : off + D, jc, :],
                    kt_sb[off : off + D, jc, :],
                    start=True,
                    stop=True,
                )
                qk = small.tile([C, C], F32, name="qk")
                nc.scalar.mul(qk[:], psa[:], 0.125)  # (d ** -0.25) ** 2 folded
                nmax = small.tile([C, 1], F32, name="nmax")
                nc.vector.reduce_max(
                    nmax[:], qk[:], axis=mybir.AxisListType.X, negate=True
                )
                ex = small.tile([C, C], F32, name="ex")
                nc.scalar.activation(
                    ex[:],
                    qk[:],
                    mybir.ActivationFunctionType.Exp,
                    bias=nmax[:],
                    scale=1.0,
                )
                esum = small.tile([C, 1], F32, name="esum")
                nc.vector.reduce_sum(esum[:], ex[:], axis=mybir.AxisListType.X)
                rsum = small.tile([C, 1], F32, name="rsum")
                nc.vector.reciprocal(rsum[:], esum[:])
                wsm = small.tile([C, C], F32, name="wsm")
                nc.vector.tensor_scalar_mul(wsm[:], ex[:], rsum[:])
                pst = pqk.tile([C, C], F32, name="pst", tag="smallps")
                nc.tensor.transpose(pst[:], wsm[:], ident[:])
                nc.vector.tensor_copy(wt_sb[:, h, :], pst[:])

            # Streaming channel-mix. Inputs ride the SP HWDGE ring, outputs
            # the ACT ring, so both directions stream concurrently. Per 2MB
            # tile: one N=512 matmul per head into a per-head PSUM bank
            # (out[c, (d, t)] via the (d, t) access pattern on xt), then a
            # per-head interleaving copy into the (t, f) staging tile.
            for rep in range(repeat):
                for it in range(TQ // TT):
                    xt = xpool.tile([C, TT, F], F32, name="xt")
                    nc.sync.dma_start(xt[:], xs[:, ts(it, TT), :])
                    ot = opool.tile([C, TT, F], F32, name="ot")
                    for h in range(H):
                        pso = pbig.tile([C, D, TT], F32, name="pso")
                        nc.tensor.matmul(
                            pso[:],
                            wt_sb[:, h, :],
                            xt[:, :, ds(D * h, D)].rearrange("k t d -> k d t"),
                            start=True,
                            stop=True,
                        )
                        nc.vector.tensor_copy(
                            ot[:, :, ds(D * h, D)],
                            pso[:].rearrange("c d t -> c t d"),
                        )
                    nc.scalar.dma_start(out[:, ts(it, TT), :], ot[:])
    nc.finalize()
    return nc


def _programs():
    if "p1" not in _CACHE:
        _CACHE["p1"] = _build_phase1()
        _CACHE["p2"] = _build_phase2()
    return _CACHE["p1"], _CACHE["p2"]


def kernel(x, Wq, bq, Wk):
    x = np.ascontiguousarray(np.asarray(x), dtype=np.float32)
    Wq = np.asarray(Wq, dtype=np.float32)
    bq = np.asarray(bq, dtype=np.float32)
    Wk = np.asarray(Wk, dtype=np.float32)
    assert x.shape == (B, C, T, F)

    nc1, nc2 = _programs()
    core_ids = list(range(NCORES))

    xs_list = []
    for i in range(NCORES):
        b, q = divmod(i, 4)
        xs_list.append(np.ascontiguousarray(x[b, :, q * TQ : (q + 1) * TQ, :]))

    r1 = run_bass_kernel_spmd(
        nc1, [{"xs": xs_list[i]} for i in range(NCORES)], core_ids, trace=TRACE
    )
    LAST_PROFILE["phase1_ns"] = r1.exec_time_ns

    xm = np.zeros((B, C, F), np.float64)
    for i in range(NCORES):
        xm[i // 4] += r1.results[i]["ps"].astype(np.float64)
    xm = (xm / T).astype(np.float32)

    xmT = [np.ascontiguousarray(xm[b].T) for b in range(B)]
    WqT = np.ascontiguousarray(Wq.T)
    WkT = np.ascontiguousarray(Wk.T)
    bqr = np.ascontiguousarray(bq.reshape(FCHUNKS, 128).T)

    in_maps = []
    for i in range(NCORES):
        b = i // 4
        in_maps.append(
            {"xs": xs_list[i], "xmt": xmT[b], "wqt": WqT, "wkt": WkT, "bqr": bqr}
        )
    r2 = run_bass_kernel_spmd(nc2, in_maps, core_ids, trace=TRACE)
    LAST_PROFILE["phase2_ns"] = r2.exec_time_ns

    out = np.empty((B, C, T, F), np.float32)
    for i in range(NCORES):
        b, q = divmod(i, 4)
        out[b, :, q * TQ : (q + 1) * TQ, :] = r2.results[i]["out"]
    return out
